# revision 14
# baseline (speedup 1.0000x reference)
"""Trainium2 Bass kernel for DNN-IVA (15-iteration ISS + per-frame MLP mask net).

Sharding: data-parallel over B (4 ways) x T (2 ways) = 8 cores.
Each core handles one batch element's half of the time frames.  The only
cross-core coupling is the per-iteration reduction over T (the ISS statistics),
reformulated so each iteration needs exactly ONE tiny pair-AllReduce (20 KB).

Math reformulation (validated vs reference): per iteration, both ISS source
steps depend on the big (C,F,T) tensors only through 8 per-(f) reductions
  q0..q3 = sum_t w_c * |Y_i|^2,   q4..q7 = sum_t w_c * Re/Im(Y1 conj(Y0))
after which the source-step updates collapse to a per-frequency 2x2 complex
matrix A applied to the two channel rows:  Y'' = A Y.

On-chip layout: f on partitions (5 chunks of 128; chunk 4 has 1 valid lane),
t on the free dimension.  Products+reductions fused via tensor_tensor_reduce;
the 2x2 apply uses scalar_tensor_tensor with per-partition coefficient APs.
"""

import os

import numpy as np

import concourse.bass as bass
import concourse.tile as tile
from concourse import bacc, mybir, masks
from concourse.bass_utils import run_bass_kernel_spmd

B, T, C, F, U = 4, 1000, 2, 513, 256
N_ITER = 15
EPS = 1e-6
N_CORES = 8
TSPLIT = 2
TL = T // TSPLIT          # 500 local frames per core
NJ = 5                    # f chunks of 128 (last has 1 valid row)
FSZ = [128, 128, 128, 128, 1]
TT_SIZES = [128, 128, 128, 116]   # t tiles covering TL=500 for load/store
FP = mybir.dt.float32
BF = mybir.dt.bfloat16
AL = mybir.AluOpType
AF = mybir.ActivationFunctionType

_CACHED = {}


def _fslice(tile_ap, j, cols):
    """AP for f-chunk j of a [128, NJ*TL]-shaped plane (cols=TL), valid lanes only."""
    return tile_ap[0 : FSZ[j], j * cols : (j + 1) * cols]


def _build():
    nc = bacc.Bacc("TRN2", target_bir_lowering=False, debug=False,
                   num_devices=N_CORES)

    xr_d = nc.dram_tensor("xr", [TL, C, F], FP, kind="ExternalInput").ap()
    xi_d = nc.dram_tensor("xi", [TL, C, F], FP, kind="ExternalInput").ap()
    w1_d = nc.dram_tensor("W1", [F, U], FP, kind="ExternalInput").ap()
    b1_d = nc.dram_tensor("b1", [U], FP, kind="ExternalInput").ap()
    w2_d = nc.dram_tensor("W2", [U, F], FP, kind="ExternalInput").ap()
    b2_d = nc.dram_tensor("b2", [F], FP, kind="ExternalInput").ap()
    yr_d = nc.dram_tensor("yr", [C, TL, F], FP, kind="ExternalOutput").ap()
    yi_d = nc.dram_tensor("yi", [C, TL, F], FP, kind="ExternalOutput").ap()

    with tile.TileContext(nc) as tc:
        _body(nc, tc, xr_d, xi_d, w1_d, b1_d, w2_d, b2_d, yr_d, yi_d)
    nc.compile()
    return nc


def _body(nc, tc, xr_d, xi_d, w1_d, b1_d, w2_d, b2_d, yr_d, yi_d):
    PLANE = NJ * TL
    with (
        tc.tile_pool(name="state", bufs=1) as st,
        tc.tile_pool(name="scr", bufs=3) as scr,
        tc.tile_pool(name="feat", bufs=3) as featp,
        tc.tile_pool(name="hpool", bufs=2) as hp,
        tc.tile_pool(name="small", bufs=12) as sm,
        tc.tile_pool(name="coef", bufs=2) as cf,
        tc.tile_pool(name="psA", bufs=2, space="PSUM") as psA,
        tc.tile_pool(name="psB", bufs=2, space="PSUM") as psB,
        tc.tile_pool(name="dram", bufs=2, space="DRAM") as dram,
        tc.tile_pool(name="outp", bufs=3) as outp,
    ):
        # ---- persistent state -------------------------------------------
        Y = [[st.tile([128, PLANE], FP, tag=f"Y{c}{p}") for p in range(2)]
             for c in range(C)]                       # [c][0]=re, [1]=im
        X0 = [st.tile([128, PLANE], FP, tag=f"X0{p}") for p in range(2)]
        A = [st.tile([128, PLANE], FP, tag=f"a{c}") for c in range(C)]
        Wm = [st.tile([128, PLANE], FP, tag=f"w{c}") for c in range(C)]
        W1t = st.tile([128, NJ * U], FP, tag="W1t")
        W2t = st.tile([128, 2 * F], FP, tag="W2t")
        b1t = st.tile([128, 2], FP, tag="b1t")
        b2t = st.tile([128, NJ], FP, tag="b2t")
        ident = st.tile([128, 128], FP, tag="ident")
        S = st.tile([128, 8 * NJ], FP, tag="S")       # quantity-major
        PB = st.tile([128, 12 * NJ], FP, tag="PB")    # projection-back stats

        masks.make_identity(nc, ident[:])

        # ---- load weights ----------------------------------------------
        for j in range(NJ):
            nc.sync.dma_start(W1t[0 : FSZ[j], j * U : (j + 1) * U],
                              w1_d[128 * j : 128 * j + FSZ[j], :])
            nc.sync.dma_start(b2t[0 : FSZ[j], j : j + 1],
                              b2_d[128 * j : 128 * j + FSZ[j]].rearrange("(p o) -> p o", o=1))
        for jc in range(2):
            nc.sync.dma_start(W2t[:, jc * F : (jc + 1) * F],
                              w2_d[128 * jc : 128 * (jc + 1), :])
            nc.sync.dma_start(b1t[:, jc : jc + 1],
                              b1_d[128 * jc : 128 * (jc + 1)].rearrange("(p o) -> p o", o=1))

        # ---- load input planes: (t,f) tiles -> PE transpose -> (f,t) ----
        for c in range(C):
            for p, src in ((0, xr_d), (1, xi_d)):
                for ti, th in enumerate(TT_SIZES):
                    it_t = scr.tile([128, F], FP, tag="ld")
                    nc.sync.dma_start(it_t[0:th, :], src[ti * 128 : ti * 128 + th, c, :])
                    for j in range(NJ):
                        fj = FSZ[j]
                        ps = psB.tile([128, 128], FP, tag="tp")
                        nc.tensor.transpose(ps[0:fj, 0:th],
                                            it_t[0:th, 128 * j : 128 * j + fj],
                                            ident[0:th, 0:th])
                        nc.scalar.copy(
                            Y[c][p][0:fj, j * TL + ti * 128 : j * TL + ti * 128 + th],
                            ps[0:fj, 0:th])
        for p in range(2):
            nc.vector.tensor_copy(X0[p][:], Y[0][p][:])

        # ---- helper groups ---------------------------------------------
        def qs(q):            # [128, NJ] AP of quantity q in S
            return S[:, q * NJ : (q + 1) * NJ]

        def mask_phase():
            for c in range(C):
                ph = [psA.tile([128, TL], FP, tag="ph") for _ in range(2)]
                for j in range(NJ):
                    fj = FSZ[j]
                    s1 = scr.tile([128, TL], FP, tag="sq")
                    s2 = scr.tile([128, TL], FP, tag="sq")
                    nc.scalar.activation(s1[0:fj, :], _fslice(Y[c][0], j, TL), AF.Square)
                    nc.scalar.activation(s2[0:fj, :], _fslice(Y[c][1], j, TL), AF.Square)
                    nc.gpsimd.tensor_add(_fslice(A[c], j, TL), s1[0:fj, :], s2[0:fj, :])
                    ft = featp.tile([128, TL], FP, tag="ft")
                    nc.scalar.activation(ft[0:fj, :], _fslice(A[c], j, TL), AF.Ln,
                                         bias=1.0)
                    for m in range(2):
                        nc.tensor.matmul(
                            ph[m][:, :],
                            W1t[0:fj, j * U + 128 * m : j * U + 128 * (m + 1)],
                            ft[0:fj, :],
                            start=(j == 0), stop=(j == NJ - 1))
                ht = hp.tile([128, 2 * TL], FP, tag="ht")
                for m in range(2):
                    nc.scalar.activation(ht[:, m * TL : (m + 1) * TL], ph[m][:, :],
                                         AF.Tanh, bias=b1t[:, m : m + 1])
                for j in range(NJ):
                    fj = FSZ[j]
                    pm = psB.tile([128, TL], FP, tag="pm")
                    for jc in range(2):
                        nc.tensor.matmul(
                            pm[0:fj, :],
                            W2t[:, jc * F + 128 * j : jc * F + 128 * j + fj],
                            ht[:, jc * TL : (jc + 1) * TL],
                            start=(jc == 0), stop=(jc == 1))
                    nc.scalar.activation(_fslice(Wm[c], j, TL), pm[0:fj, :],
                                         AF.Sigmoid, bias=b2t[0:fj, j : j + 1])

        def stats_phase():
            for j in range(NJ):
                fj = FSZ[j]
                y0r, y0i = _fslice(Y[0][0], j, TL), _fslice(Y[0][1], j, TL)
                y1r, y1i = _fslice(Y[1][0], j, TL), _fslice(Y[1][1], j, TL)
                m1 = scr.tile([128, TL], FP, tag="pp")
                m2 = scr.tile([128, TL], FP, tag="pp")
                pr = scr.tile([128, TL], FP, tag="pr")
                nc.vector.tensor_mul(m1[0:fj, :], y1r, y0r)
                nc.vector.tensor_mul(m2[0:fj, :], y1i, y0i)
                nc.vector.tensor_add(pr[0:fj, :], m1[0:fj, :], m2[0:fj, :])
                m3 = scr.tile([128, TL], FP, tag="pp")
                m4 = scr.tile([128, TL], FP, tag="pp")
                pi = scr.tile([128, TL], FP, tag="pi")
                nc.gpsimd.tensor_mul(m3[0:fj, :], y1i, y0r)
                nc.gpsimd.tensor_mul(m4[0:fj, :], y1r, y0i)
                nc.gpsimd.tensor_sub(pi[0:fj, :], m3[0:fj, :], m4[0:fj, :])
                srcs = [(Wm[0], A[0], 0), (Wm[1], A[0], 1),
                        (Wm[0], A[1], 2), (Wm[1], A[1], 3)]
                for wt, at, q in srcs:
                    dump = scr.tile([128, TL], FP, tag="dump")
                    nc.vector.tensor_tensor_reduce(
                        dump[0:fj, :], _fslice(wt, j, TL), _fslice(at, j, TL),
                        1.0, 0.0, AL.mult, AL.add,
                        S[0:fj, q * NJ + j : q * NJ + j + 1])
                for wt, pt, q in [(Wm[0], pr, 4), (Wm[0], pi, 5),
                                  (Wm[1], pr, 6), (Wm[1], pi, 7)]:
                    dump = scr.tile([128, TL], FP, tag="dump")
                    nc.vector.tensor_tensor_reduce(
                        dump[0:fj, :], _fslice(wt, j, TL), pt[0:fj, :],
                        1.0, 0.0, AL.mult, AL.add,
                        S[0:fj, q * NJ + j : q * NJ + j + 1])

        def allreduce(tile_t, ncols):
            bi = dram.tile([128, ncols], FP, tag="cin")
            bo = dram.tile([128, ncols], FP, tag="cout")
            nc.sync.dma_start(bi[:], tile_t[:, 0:ncols])
            nc.gpsimd.collective_compute(
                "AllReduce", AL.add,
                replica_groups=[[0, 1], [2, 3], [4, 5], [6, 7]],
                ins=[bi.opt()], outs=[bo.opt()])
            nc.sync.dma_start(tile_t[:, 0:ncols], bo[:])

        def smalls():
            """Per-(f) coefficient algebra on [128, NJ] tiles."""
            def t():
                return sm.tile([128, NJ], FP, tag="smt")

            def c(name):
                return cf.tile([128, NJ], FP, tag=name, name=name)
            invT = 1.0 / float(T)
            d0, r0 = t(), t()
            alpha = c("alpha")
            nc.vector.tensor_scalar(d0[:], qs(0), invT, EPS, AL.mult, AL.max)
            nc.vector.reciprocal(r0[:], d0[:])
            nc.scalar.activation(alpha[:], r0[:], AF.Sqrt)
            d1, r1 = t(), t()
            nc.vector.tensor_scalar(d1[:], qs(1), EPS, None, AL.max)
            nc.vector.reciprocal(r1[:], d1[:])
            vr = t()
            vi, nvr, nvi = c("vi"), c("nvr"), c("nvi")
            nc.vector.tensor_mul(vr[:], qs(6), r1[:])
            nc.vector.tensor_mul(vi[:], qs(7), r1[:])
            nc.vector.tensor_scalar_mul(nvr[:], vr[:], -1.0)
            nc.vector.tensor_scalar_mul(nvi[:], vi[:], -1.0)
            m2, u = t(), t()
            nc.vector.tensor_mul(m2[:], vr[:], vr[:])
            nc.vector.scalar_tensor_tensor(u[:], vi[:], 1.0, vi[:], AL.mult, AL.mult)
            nc.vector.tensor_add(m2[:], m2[:], u[:])
            # den0' = q2 - 2(vr q4 + vi q5) + m2 q0 ; den1' likewise with q6,q7,q1,q3
            def denp(qa, qb, qden, qs11):
                x1, x2, e = t(), t(), t()
                nc.vector.tensor_mul(x1[:], vr[:], qa)
                nc.vector.scalar_tensor_tensor(x2[:], vi[:], 1.0, qb, AL.mult, AL.mult)
                nc.vector.tensor_add(x1[:], x1[:], x2[:])
                nc.vector.tensor_mul(e[:], m2[:], qden)
                o = t()
                nc.vector.scalar_tensor_tensor(o[:], x1[:], -2.0, qs11, AL.mult, AL.add)
                nc.vector.tensor_add(o[:], o[:], e[:])
                return o
            den0p = denp(qs(4), qs(5), qs(0), qs(2))
            den1p = denp(qs(6), qs(7), qs(1), qs(3))
            dm, rdm = t(), t()
            nc.vector.tensor_scalar(dm[:], den0p[:], EPS, None, AL.max)
            nc.vector.reciprocal(rdm[:], dm[:])
            # v1 = alpha*((q4,-q5) - conj(v) q0) / den0p
            v1r, tA, tB = t(), t(), t()
            v1i, nv1r, nv1i = c("v1i"), c("nv1r"), c("nv1i")
            nc.vector.tensor_mul(tA[:], vr[:], qs(0))
            nc.vector.tensor_sub(tA[:], qs(4), tA[:])
            nc.vector.tensor_mul(tA[:], tA[:], alpha[:])
            nc.vector.tensor_mul(v1r[:], tA[:], rdm[:])
            nc.vector.tensor_mul(tB[:], vi[:], qs(0))
            nc.vector.tensor_sub(tB[:], tB[:], qs(5))
            nc.vector.tensor_mul(tB[:], tB[:], alpha[:])
            nc.vector.tensor_mul(v1i[:], tB[:], rdm[:])
            nc.vector.tensor_scalar_mul(nv1r[:], v1r[:], -1.0)
            nc.vector.tensor_scalar_mul(nv1i[:], v1i[:], -1.0)
            db, rb = t(), t()
            beta = c("beta")
            nc.vector.tensor_scalar(db[:], den1p[:], invT, EPS, AL.mult, AL.max)
            nc.vector.reciprocal(rb[:], db[:])
            nc.scalar.activation(beta[:], rb[:], AF.Sqrt)
            return alpha, beta, vi, nvr, nvi, v1i, nv1r, nv1i

        def apply_phase(alpha, beta, vi, nvr, nvi, v1i, nv1r, nv1i):
            for j in range(NJ):
                fj = FSZ[j]
                y0r, y0i = _fslice(Y[0][0], j, TL), _fslice(Y[0][1], j, TL)
                y1r, y1i = _fslice(Y[1][0], j, TL), _fslice(Y[1][1], j, TL)
                def c_(ct):
                    return ct[0:fj, j : j + 1]
                t1 = scr.tile([128, TL], FP, tag="ap")
                y1pr = scr.tile([128, TL], FP, tag="y1p")
                nc.vector.scalar_tensor_tensor(t1[0:fj, :], y0r, c_(nvr), y1r,
                                               AL.mult, AL.add)
                nc.vector.scalar_tensor_tensor(y1pr[0:fj, :], y0i, c_(vi), t1[0:fj, :],
                                               AL.mult, AL.add)
                t2 = scr.tile([128, TL], FP, tag="ap")
                y1pi = scr.tile([128, TL], FP, tag="y1p")
                nc.vector.scalar_tensor_tensor(t2[0:fj, :], y0i, c_(nvr), y1i,
                                               AL.mult, AL.add)
                nc.vector.scalar_tensor_tensor(y1pi[0:fj, :], y0r, c_(nvi), t2[0:fj, :],
                                               AL.mult, AL.add)
                s1 = scr.tile([128, TL], FP, tag="ap")
                s2 = scr.tile([128, TL], FP, tag="ap")
                nc.scalar.mul(s1[0:fj, :], y0r, c_(alpha))
                nc.scalar.mul(s2[0:fj, :], y0i, c_(alpha))
                t3 = scr.tile([128, TL], FP, tag="ap")
                nc.vector.scalar_tensor_tensor(t3[0:fj, :], y1pr[0:fj, :], c_(nv1r),
                                               s1[0:fj, :], AL.mult, AL.add)
                nc.vector.scalar_tensor_tensor(y0r, y1pi[0:fj, :], c_(v1i),
                                               t3[0:fj, :], AL.mult, AL.add)
                t4 = scr.tile([128, TL], FP, tag="ap")
                nc.vector.scalar_tensor_tensor(t4[0:fj, :], y1pi[0:fj, :], c_(nv1r),
                                               s2[0:fj, :], AL.mult, AL.add)
                nc.vector.scalar_tensor_tensor(y0i, y1pr[0:fj, :], c_(nv1i),
                                               t4[0:fj, :], AL.mult, AL.add)
                nc.scalar.mul(y1r, y1pr[0:fj, :], c_(beta))
                nc.scalar.mul(y1i, y1pi[0:fj, :], c_(beta))

        # ---- main loop ---------------------------------------------------
        n_it = int(os.environ.get("KITERS", str(N_ITER)))
        do_cc = os.environ.get("KCC", "1") == "1"
        do_pb = os.environ.get("KPB", "1") == "1"
        do_mask = os.environ.get("KMASK", "1") == "1"
        do_stats = os.environ.get("KSTATS", "1") == "1"
        do_apply = os.environ.get("KAPPLY", "1") == "1"
        for _ in range(n_it):
            if do_mask:
                mask_phase()
            if do_stats:
                stats_phase()
            if do_cc:
                allreduce(S, 8 * NJ)
            if do_apply:
                coefs = smalls()
                apply_phase(*coefs)

        # ---- projection back --------------------------------------------
        if not do_pb:
            pb_skip = True
        for j in ([] if not do_pb else range(NJ)):
            fj = FSZ[j]
            for c in range(C):
                pairs = [(Y[c][0], X0[0]), (Y[c][1], X0[1]),
                         (Y[c][0], X0[1]), (Y[c][1], X0[0]),
                         (Y[c][0], Y[c][0]), (Y[c][1], Y[c][1])]
                for qi, (ta, tb) in enumerate(pairs):
                    q = c * 6 + qi
                    dump = scr.tile([128, TL], FP, tag="dump")
                    nc.vector.tensor_tensor_reduce(
                        dump[0:fj, :], _fslice(ta, j, TL), _fslice(tb, j, TL),
                        1.0, 0.0, AL.mult, AL.add,
                        PB[0:fj, q * NJ + j : q * NJ + j + 1])
        if do_pb:
            allreduce(PB, 12 * NJ)

        def pbq(q):
            return PB[:, q * NJ : (q + 1) * NJ]

        for c in ([] if not do_pb else range(C)):
            g = [pbq(c * 6 + i) for i in range(6)]
            numr = sm.tile([128, NJ], FP, tag="pbs")
            numi = sm.tile([128, NJ], FP, tag="pbs")
            den = sm.tile([128, NJ], FP, tag="pbs")
            rc = sm.tile([128, NJ], FP, tag="pbs")
            cr = sm.tile([128, NJ], FP, tag=f"cr{c}")
            ci = sm.tile([128, NJ], FP, tag=f"ci{c}")
            nci = sm.tile([128, NJ], FP, tag=f"nci{c}")
            nc.vector.tensor_add(numr[:], g[0], g[1])
            nc.vector.tensor_sub(numi[:], g[2], g[3])
            nc.vector.tensor_add(den[:], g[4], g[5])
            nc.vector.tensor_scalar(den[:], den[:], EPS, None, AL.max)
            nc.vector.reciprocal(rc[:], den[:])
            nc.vector.tensor_mul(cr[:], numr[:], rc[:])
            nc.vector.tensor_mul(ci[:], numi[:], rc[:])
            nc.vector.tensor_scalar_mul(nci[:], ci[:], -1.0)
            for j in range(NJ):
                fj = FSZ[j]
                ycr, yci = _fslice(Y[c][0], j, TL), _fslice(Y[c][1], j, TL)
                s1 = scr.tile([128, TL], FP, tag="ap")
                s2 = scr.tile([128, TL], FP, tag="ap")
                tr = scr.tile([128, TL], FP, tag="ap")
                nc.scalar.mul(s1[0:fj, :], ycr, cr[0:fj, j : j + 1])
                nc.scalar.mul(s2[0:fj, :], yci, cr[0:fj, j : j + 1])
                # new_re = cr*ycr - ci*yci ; new_im = cr*yci + ci*ycr
                nc.vector.scalar_tensor_tensor(tr[0:fj, :], yci, nci[0:fj, j : j + 1],
                                               s1[0:fj, :], AL.mult, AL.add)
                nc.vector.scalar_tensor_tensor(yci, ycr, ci[0:fj, j : j + 1],
                                               s2[0:fj, :], AL.mult, AL.add)
                nc.vector.tensor_copy(ycr, tr[0:fj, :])

        # ---- write output: transpose back to (t,f), DMA out -------------
        for c in range(C):
            for p, dst in ((0, yr_d), (1, yi_d)):
                for ti, th in enumerate(TT_SIZES):
                    ot = outp.tile([128, F], FP, tag="ot")
                    for j in range(NJ):
                        fj = FSZ[j]
                        ps = psB.tile([128, 128], FP, tag="tp")
                        nc.tensor.transpose(
                            ps[0:th, 0:fj],
                            Y[c][p][0:fj, j * TL + ti * 128 : j * TL + ti * 128 + th],
                            ident[0:fj, 0:fj])
                        nc.scalar.copy(ot[0:th, 128 * j : 128 * j + fj],
                                       ps[0:th, 0:fj])
                    nc.sync.dma_start(dst[c, ti * 128 : ti * 128 + th, :],
                                      ot[0:th, :])


def _in_maps(inputs):
    data_real = np.asarray(inputs["data_real"], dtype=np.float32)
    data_imag = np.asarray(inputs["data_imag"], dtype=np.float32)
    W1 = np.asarray(inputs["W1"], dtype=np.float32)
    b1 = np.asarray(inputs["b1"], dtype=np.float32)
    W2 = np.asarray(inputs["W2"], dtype=np.float32)
    b2 = np.asarray(inputs["b2"], dtype=np.float32)
    in_maps = []
    for k in range(N_CORES):
        b, th = k // TSPLIT, k % TSPLIT
        sl = slice(th * TL, (th + 1) * TL)
        in_maps.append({
            "xr": np.ascontiguousarray(data_real[b, sl]),
            "xi": np.ascontiguousarray(data_imag[b, sl]),
            "W1": W1, "b1": b1, "W2": W2, "b2": b2,
        })
    return in_maps


def kernel(**inputs):
    if "nc" not in _CACHED:
        _CACHED["nc"] = _build()
    nc = _CACHED["nc"]
    in_maps = _in_maps(inputs)
    res = run_bass_kernel_spmd(nc, in_maps, list(range(N_CORES)))
    out = np.empty((C, B, T, F), dtype=np.complex64)
    for k in range(N_CORES):
        b, th = k // TSPLIT, k % TSPLIT
        r = res.results[k]["yr"]
        i = res.results[k]["yi"]
        out[:, b, th * TL : (th + 1) * TL, :] = r + 1j * i
    return out


if __name__ == "__main__":
    rng = np.random.default_rng(0)
    ins = {
        "data_real": rng.standard_normal((B, T, C, F), dtype=np.float32),
        "data_imag": rng.standard_normal((B, T, C, F), dtype=np.float32),
        "ilens": np.full((B,), T, dtype=np.int32),
        "W1": rng.standard_normal((F, U), dtype=np.float32) / np.sqrt(F),
        "b1": np.zeros((U,), dtype=np.float32),
        "W2": rng.standard_normal((U, F), dtype=np.float32) / np.sqrt(U),
        "b2": np.zeros((F,), dtype=np.float32),
    }
    out = kernel(**ins)
    print("kernel ran", out.shape, out.dtype, np.abs(out).mean())


# revision 17
# speedup vs baseline: 1.0612x; 1.0612x over previous
"""Trainium2 Bass kernel for DNN-IVA (15-iteration ISS + per-frame MLP mask net).

Sharding: data-parallel over B (4 ways) x T (2 ways) = 8 cores.
Each core handles one batch element's half of the time frames.  The only
cross-core coupling is the per-iteration reduction over T (the ISS statistics),
reformulated so each iteration needs exactly ONE tiny pair-AllReduce (20 KB).

Math reformulation (validated vs reference): per iteration, both ISS source
steps depend on the big (C,F,T) tensors only through 8 per-(f) reductions
  q0..q3 = sum_t w_c * |Y_i|^2,   q4..q7 = sum_t w_c * Re/Im(Y1 conj(Y0))
after which the source-step updates collapse to a per-frequency 2x2 complex
matrix A applied to the two channel rows:  Y'' = A Y.

On-chip layout: f on partitions (5 chunks of 128; chunk 4 has 1 valid lane),
t on the free dimension.  Products+reductions fused via tensor_tensor_reduce;
the 2x2 apply uses scalar_tensor_tensor with per-partition coefficient APs.
"""

import os

import numpy as np

import concourse.bass as bass
import concourse.tile as tile
from concourse import bacc, mybir, masks
from concourse.bass_utils import run_bass_kernel_spmd

B, T, C, F, U = 4, 1000, 2, 513, 256
N_ITER = 15
EPS = 1e-6
N_CORES = 8
TSPLIT = 2
TL = T // TSPLIT          # 500 local frames per core
NJ = 5                    # f chunks of 128 (last has 1 valid row)
FSZ = [128, 128, 128, 128, 1]
TT_SIZES = [128, 128, 128, 116]   # t tiles covering TL=500 for load/store
FP = mybir.dt.float32
BF = mybir.dt.bfloat16
AL = mybir.AluOpType
AF = mybir.ActivationFunctionType

_CACHED = {}


def _fslice(tile_ap, j, cols):
    """AP for f-chunk j of a [128, NJ*TL]-shaped plane (cols=TL), valid lanes only."""
    return tile_ap[0 : FSZ[j], j * cols : (j + 1) * cols]


def _build():
    nc = bacc.Bacc("TRN2", target_bir_lowering=False, debug=False,
                   num_devices=N_CORES)

    xr_d = nc.dram_tensor("xr", [TL, C, F], FP, kind="ExternalInput").ap()
    xi_d = nc.dram_tensor("xi", [TL, C, F], FP, kind="ExternalInput").ap()
    w1_d = nc.dram_tensor("W1", [F, U], FP, kind="ExternalInput").ap()
    b1_d = nc.dram_tensor("b1", [U], FP, kind="ExternalInput").ap()
    w2_d = nc.dram_tensor("W2", [U, F], FP, kind="ExternalInput").ap()
    b2_d = nc.dram_tensor("b2", [F], FP, kind="ExternalInput").ap()
    yr_d = nc.dram_tensor("yr", [C, TL, F], FP, kind="ExternalOutput").ap()
    yi_d = nc.dram_tensor("yi", [C, TL, F], FP, kind="ExternalOutput").ap()

    with tile.TileContext(nc) as tc:
        _body(nc, tc, xr_d, xi_d, w1_d, b1_d, w2_d, b2_d, yr_d, yi_d)
    nc.compile()
    return nc


def _body(nc, tc, xr_d, xi_d, w1_d, b1_d, w2_d, b2_d, yr_d, yi_d):
    PLANE = NJ * TL
    with (
        tc.tile_pool(name="state", bufs=1) as st,
        tc.tile_pool(name="scr", bufs=3) as scr,
        tc.tile_pool(name="feat", bufs=3) as featp,
        tc.tile_pool(name="hpool", bufs=2) as hp,
        tc.tile_pool(name="small", bufs=12) as sm,
        tc.tile_pool(name="coef", bufs=2) as cf,
        tc.tile_pool(name="psA", bufs=2, space="PSUM") as psA,
        tc.tile_pool(name="psB", bufs=2, space="PSUM") as psB,
        tc.tile_pool(name="dram", bufs=2, space="DRAM") as dram,
        tc.tile_pool(name="outp", bufs=3) as outp,
    ):
        # ---- persistent state -------------------------------------------
        Y = [[st.tile([128, PLANE], FP, tag=f"Y{c}{p}") for p in range(2)]
             for c in range(C)]                       # [c][0]=re, [1]=im
        X0 = [st.tile([128, PLANE], FP, tag=f"X0{p}") for p in range(2)]
        A = [st.tile([128, PLANE], FP, tag=f"a{c}") for c in range(C)]
        Wm = [st.tile([128, PLANE], FP, tag=f"w{c}") for c in range(C)]
        W1t = st.tile([128, NJ * U], FP, tag="W1t")
        W2t = st.tile([128, 2 * F], FP, tag="W2t")
        b1t = st.tile([128, 2], FP, tag="b1t")
        b2t = st.tile([128, NJ], FP, tag="b2t")
        ident = st.tile([128, 128], FP, tag="ident")
        S = st.tile([128, 8 * NJ], FP, tag="S")       # quantity-major
        PB = st.tile([128, 12 * NJ], FP, tag="PB")    # projection-back stats

        masks.make_identity(nc, ident[:])

        # ---- load weights ----------------------------------------------
        for j in range(NJ):
            nc.sync.dma_start(W1t[0 : FSZ[j], j * U : (j + 1) * U],
                              w1_d[128 * j : 128 * j + FSZ[j], :])
            nc.sync.dma_start(b2t[0 : FSZ[j], j : j + 1],
                              b2_d[128 * j : 128 * j + FSZ[j]].rearrange("(p o) -> p o", o=1))
        for jc in range(2):
            nc.sync.dma_start(W2t[:, jc * F : (jc + 1) * F],
                              w2_d[128 * jc : 128 * (jc + 1), :])
            nc.sync.dma_start(b1t[:, jc : jc + 1],
                              b1_d[128 * jc : 128 * (jc + 1)].rearrange("(p o) -> p o", o=1))

        # ---- load input planes: (t,f) tiles -> PE transpose -> (f,t) ----
        for c in range(C):
            for p, src in ((0, xr_d), (1, xi_d)):
                for ti, th in enumerate(TT_SIZES):
                    it_t = scr.tile([128, F], FP, tag="ld")
                    nc.sync.dma_start(it_t[0:th, :], src[ti * 128 : ti * 128 + th, c, :])
                    for j in range(NJ):
                        fj = FSZ[j]
                        ps = psB.tile([128, 128], FP, tag="tp")
                        nc.tensor.transpose(ps[0:fj, 0:th],
                                            it_t[0:th, 128 * j : 128 * j + fj],
                                            ident[0:th, 0:th])
                        nc.scalar.copy(
                            Y[c][p][0:fj, j * TL + ti * 128 : j * TL + ti * 128 + th],
                            ps[0:fj, 0:th])
        for p in range(2):
            nc.vector.tensor_copy(X0[p][:], Y[0][p][:])

        # ---- helper groups ---------------------------------------------
        def qs(q):            # [128, NJ] AP of quantity q in S
            return S[:, q * NJ : (q + 1) * NJ]

        def mask_phase():
            for c in range(C):
                ph = [psA.tile([128, TL], FP, tag="ph") for _ in range(2)]
                for j in range(NJ):
                    fj = FSZ[j]
                    s1 = scr.tile([128, TL], FP, tag="sq")
                    s2 = scr.tile([128, TL], FP, tag="sq")
                    nc.scalar.activation(s1[0:fj, :], _fslice(Y[c][0], j, TL), AF.Square)
                    nc.scalar.activation(s2[0:fj, :], _fslice(Y[c][1], j, TL), AF.Square)
                    nc.gpsimd.tensor_add(_fslice(A[c], j, TL), s1[0:fj, :], s2[0:fj, :])
                    ft = featp.tile([128, TL], FP, tag="ft")
                    nc.scalar.activation(ft[0:fj, :], _fslice(A[c], j, TL), AF.Ln,
                                         bias=1.0)
                    for m in range(2):
                        nc.tensor.matmul(
                            ph[m][:, :],
                            W1t[0:fj, j * U + 128 * m : j * U + 128 * (m + 1)],
                            ft[0:fj, :],
                            start=(j == 0), stop=(j == NJ - 1))
                ht = hp.tile([128, 2 * TL], FP, tag="ht")
                for m in range(2):
                    nc.scalar.activation(ht[:, m * TL : (m + 1) * TL], ph[m][:, :],
                                         AF.Tanh, bias=b1t[:, m : m + 1])
                for j in range(NJ):
                    fj = FSZ[j]
                    pm = psB.tile([128, TL], FP, tag="pm")
                    for jc in range(2):
                        nc.tensor.matmul(
                            pm[0:fj, :],
                            W2t[:, jc * F + 128 * j : jc * F + 128 * j + fj],
                            ht[:, jc * TL : (jc + 1) * TL],
                            start=(jc == 0), stop=(jc == 1))
                    nc.scalar.activation(_fslice(Wm[c], j, TL), pm[0:fj, :],
                                         AF.Sigmoid, bias=b2t[0:fj, j : j + 1])

        def stats_phase():
            for j in range(NJ):
                fj = FSZ[j]
                y0r, y0i = _fslice(Y[0][0], j, TL), _fslice(Y[0][1], j, TL)
                y1r, y1i = _fslice(Y[1][0], j, TL), _fslice(Y[1][1], j, TL)
                m1 = scr.tile([128, TL], FP, tag="pp")
                m2 = scr.tile([128, TL], FP, tag="pp")
                pr = scr.tile([128, TL], FP, tag="pr")
                nc.vector.tensor_mul(m1[0:fj, :], y1r, y0r)
                nc.vector.tensor_mul(m2[0:fj, :], y1i, y0i)
                nc.vector.tensor_add(pr[0:fj, :], m1[0:fj, :], m2[0:fj, :])
                m3 = scr.tile([128, TL], FP, tag="pp")
                m4 = scr.tile([128, TL], FP, tag="pp")
                pi = scr.tile([128, TL], FP, tag="pi")
                nc.gpsimd.tensor_mul(m3[0:fj, :], y1i, y0r)
                nc.gpsimd.tensor_mul(m4[0:fj, :], y1r, y0i)
                nc.gpsimd.tensor_sub(pi[0:fj, :], m3[0:fj, :], m4[0:fj, :])
                srcs = [(Wm[0], A[0], 0), (Wm[1], A[0], 1),
                        (Wm[0], A[1], 2), (Wm[1], A[1], 3)]
                for wt, at, q in srcs:
                    dump = scr.tile([128, TL], FP, tag="dump")
                    nc.vector.tensor_tensor_reduce(
                        dump[0:fj, :], _fslice(wt, j, TL), _fslice(at, j, TL),
                        1.0, 0.0, AL.mult, AL.add,
                        S[0:fj, q * NJ + j : q * NJ + j + 1])
                for wt, pt, q in [(Wm[0], pr, 4), (Wm[0], pi, 5),
                                  (Wm[1], pr, 6), (Wm[1], pi, 7)]:
                    dump = scr.tile([128, TL], FP, tag="dump")
                    nc.vector.tensor_tensor_reduce(
                        dump[0:fj, :], _fslice(wt, j, TL), pt[0:fj, :],
                        1.0, 0.0, AL.mult, AL.add,
                        S[0:fj, q * NJ + j : q * NJ + j + 1])

        def allreduce(tile_t, ncols):
            bi = dram.tile([128, ncols], FP, tag="cin")
            bo = dram.tile([128, ncols], FP, tag="cout")
            nc.sync.dma_start(bi[:], tile_t[:, 0:ncols])
            nc.gpsimd.collective_compute(
                "AllReduce", AL.add,
                replica_groups=[[0, 1], [2, 3], [4, 5], [6, 7]],
                ins=[bi.opt()], outs=[bo.opt()])
            nc.sync.dma_start(tile_t[:, 0:ncols], bo[:])

        def smalls():
            """Per-(f) coefficient algebra on [128, NJ] tiles."""
            def t():
                return sm.tile([128, NJ], FP, tag="smt")

            def c(name):
                return cf.tile([128, NJ], FP, tag=name, name=name)
            invT = 1.0 / float(T)
            d0, r0 = t(), t()
            alpha = c("alpha")
            nc.vector.tensor_scalar(d0[:], qs(0), invT, EPS, AL.mult, AL.max)
            nc.vector.reciprocal(r0[:], d0[:])
            nc.scalar.activation(alpha[:], r0[:], AF.Sqrt)
            d1, r1 = t(), t()
            nc.vector.tensor_scalar(d1[:], qs(1), EPS, None, AL.max)
            nc.vector.reciprocal(r1[:], d1[:])
            vr = t()
            vi, nvr, nvi = c("vi"), c("nvr"), c("nvi")
            nc.vector.tensor_mul(vr[:], qs(6), r1[:])
            nc.vector.tensor_mul(vi[:], qs(7), r1[:])
            nc.vector.tensor_scalar_mul(nvr[:], vr[:], -1.0)
            nc.vector.tensor_scalar_mul(nvi[:], vi[:], -1.0)
            m2, u = t(), t()
            nc.vector.tensor_mul(m2[:], vr[:], vr[:])
            nc.vector.scalar_tensor_tensor(u[:], vi[:], 1.0, vi[:], AL.mult, AL.mult)
            nc.vector.tensor_add(m2[:], m2[:], u[:])
            # den0' = q2 - 2(vr q4 + vi q5) + m2 q0 ; den1' likewise with q6,q7,q1,q3
            def denp(qa, qb, qden, qs11):
                x1, x2, e = t(), t(), t()
                nc.vector.tensor_mul(x1[:], vr[:], qa)
                nc.vector.scalar_tensor_tensor(x2[:], vi[:], 1.0, qb, AL.mult, AL.mult)
                nc.vector.tensor_add(x1[:], x1[:], x2[:])
                nc.vector.tensor_mul(e[:], m2[:], qden)
                o = t()
                nc.vector.scalar_tensor_tensor(o[:], x1[:], -2.0, qs11, AL.mult, AL.add)
                nc.vector.tensor_add(o[:], o[:], e[:])
                return o
            den0p = denp(qs(4), qs(5), qs(0), qs(2))
            den1p = denp(qs(6), qs(7), qs(1), qs(3))
            dm, rdm = t(), t()
            nc.vector.tensor_scalar(dm[:], den0p[:], EPS, None, AL.max)
            nc.vector.reciprocal(rdm[:], dm[:])
            # v1 = alpha*((q4,-q5) - conj(v) q0) / den0p
            v1r, tA, tB = t(), t(), t()
            v1i, nv1r, nv1i = c("v1i"), c("nv1r"), c("nv1i")
            nc.vector.tensor_mul(tA[:], vr[:], qs(0))
            nc.vector.tensor_sub(tA[:], qs(4), tA[:])
            nc.vector.tensor_mul(tA[:], tA[:], alpha[:])
            nc.vector.tensor_mul(v1r[:], tA[:], rdm[:])
            nc.vector.tensor_mul(tB[:], vi[:], qs(0))
            nc.vector.tensor_sub(tB[:], tB[:], qs(5))
            nc.vector.tensor_mul(tB[:], tB[:], alpha[:])
            nc.vector.tensor_mul(v1i[:], tB[:], rdm[:])
            nc.vector.tensor_scalar_mul(nv1r[:], v1r[:], -1.0)
            nc.vector.tensor_scalar_mul(nv1i[:], v1i[:], -1.0)
            db, rb = t(), t()
            beta = c("beta")
            nc.vector.tensor_scalar(db[:], den1p[:], invT, EPS, AL.mult, AL.max)
            nc.vector.reciprocal(rb[:], db[:])
            nc.scalar.activation(beta[:], rb[:], AF.Sqrt)
            return alpha, beta, vi, nvr, nvi, v1i, nv1r, nv1i

        def apply_phase(alpha, beta, vi, nvr, nvi, v1i, nv1r, nv1i):
            for j in range(NJ):
                fj = FSZ[j]
                y0r, y0i = _fslice(Y[0][0], j, TL), _fslice(Y[0][1], j, TL)
                y1r, y1i = _fslice(Y[1][0], j, TL), _fslice(Y[1][1], j, TL)
                def c_(ct):
                    return ct[0:fj, j : j + 1]
                t1 = scr.tile([128, TL], FP, tag="ap")
                y1pr = scr.tile([128, TL], FP, tag="y1p")
                nc.vector.scalar_tensor_tensor(t1[0:fj, :], y0r, c_(nvr), y1r,
                                               AL.mult, AL.add)
                nc.vector.scalar_tensor_tensor(y1pr[0:fj, :], y0i, c_(vi), t1[0:fj, :],
                                               AL.mult, AL.add)
                t2 = scr.tile([128, TL], FP, tag="ap")
                y1pi = scr.tile([128, TL], FP, tag="y1p")
                nc.vector.scalar_tensor_tensor(t2[0:fj, :], y0i, c_(nvr), y1i,
                                               AL.mult, AL.add)
                nc.vector.scalar_tensor_tensor(y1pi[0:fj, :], y0r, c_(nvi), t2[0:fj, :],
                                               AL.mult, AL.add)
                s1 = scr.tile([128, TL], FP, tag="ap")
                s2 = scr.tile([128, TL], FP, tag="ap")
                nc.scalar.mul(s1[0:fj, :], y0r, c_(alpha))
                nc.scalar.mul(s2[0:fj, :], y0i, c_(alpha))
                t3 = scr.tile([128, TL], FP, tag="ap")
                nc.vector.scalar_tensor_tensor(t3[0:fj, :], y1pr[0:fj, :], c_(nv1r),
                                               s1[0:fj, :], AL.mult, AL.add)
                nc.vector.scalar_tensor_tensor(y0r, y1pi[0:fj, :], c_(v1i),
                                               t3[0:fj, :], AL.mult, AL.add)
                t4 = scr.tile([128, TL], FP, tag="ap")
                nc.vector.scalar_tensor_tensor(t4[0:fj, :], y1pi[0:fj, :], c_(nv1r),
                                               s2[0:fj, :], AL.mult, AL.add)
                nc.vector.scalar_tensor_tensor(y0i, y1pr[0:fj, :], c_(nv1i),
                                               t4[0:fj, :], AL.mult, AL.add)
                nc.scalar.mul(y1r, y1pr[0:fj, :], c_(beta))
                nc.scalar.mul(y1i, y1pi[0:fj, :], c_(beta))

        # ---- main loop ---------------------------------------------------
        n_it = int(os.environ.get("KITERS", str(N_ITER)))
        do_cc = os.environ.get("KCC", "1") == "1"
        do_pb = os.environ.get("KPB", "1") == "1"
        do_mask = os.environ.get("KMASK", "1") == "1"
        do_stats = os.environ.get("KSTATS", "1") == "1"
        do_apply = os.environ.get("KAPPLY", "1") == "1"
        for _ in range(n_it):
            if do_mask:
                mask_phase()
            if do_stats:
                stats_phase()
            if do_cc:
                allreduce(S, 8 * NJ)
            if do_apply:
                coefs = smalls()
                apply_phase(*coefs)

        # ---- projection back --------------------------------------------
        if not do_pb:
            pb_skip = True
        for j in ([] if not do_pb else range(NJ)):
            fj = FSZ[j]
            for c in range(C):
                pairs = [(Y[c][0], X0[0]), (Y[c][1], X0[1]),
                         (Y[c][0], X0[1]), (Y[c][1], X0[0]),
                         (Y[c][0], Y[c][0]), (Y[c][1], Y[c][1])]
                for qi, (ta, tb) in enumerate(pairs):
                    q = c * 6 + qi
                    dump = scr.tile([128, TL], FP, tag="dump")
                    nc.vector.tensor_tensor_reduce(
                        dump[0:fj, :], _fslice(ta, j, TL), _fslice(tb, j, TL),
                        1.0, 0.0, AL.mult, AL.add,
                        PB[0:fj, q * NJ + j : q * NJ + j + 1])
        if do_pb:
            allreduce(PB, 12 * NJ)

        def pbq(q):
            return PB[:, q * NJ : (q + 1) * NJ]

        for c in ([] if not do_pb else range(C)):
            g = [pbq(c * 6 + i) for i in range(6)]
            numr = sm.tile([128, NJ], FP, tag="pbs")
            numi = sm.tile([128, NJ], FP, tag="pbs")
            den = sm.tile([128, NJ], FP, tag="pbs")
            rc = sm.tile([128, NJ], FP, tag="pbs")
            cr = sm.tile([128, NJ], FP, tag=f"cr{c}")
            ci = sm.tile([128, NJ], FP, tag=f"ci{c}")
            nci = sm.tile([128, NJ], FP, tag=f"nci{c}")
            nc.vector.tensor_add(numr[:], g[0], g[1])
            nc.vector.tensor_sub(numi[:], g[2], g[3])
            nc.vector.tensor_add(den[:], g[4], g[5])
            nc.vector.tensor_scalar(den[:], den[:], EPS, None, AL.max)
            nc.vector.reciprocal(rc[:], den[:])
            nc.vector.tensor_mul(cr[:], numr[:], rc[:])
            nc.vector.tensor_mul(ci[:], numi[:], rc[:])
            nc.vector.tensor_scalar_mul(nci[:], ci[:], -1.0)
            for j in range(NJ):
                fj = FSZ[j]
                ycr, yci = _fslice(Y[c][0], j, TL), _fslice(Y[c][1], j, TL)
                s1 = scr.tile([128, TL], FP, tag="ap")
                s2 = scr.tile([128, TL], FP, tag="ap")
                tr = scr.tile([128, TL], FP, tag="ap")
                nc.scalar.mul(s1[0:fj, :], ycr, cr[0:fj, j : j + 1])
                nc.scalar.mul(s2[0:fj, :], yci, cr[0:fj, j : j + 1])
                # new_re = cr*ycr - ci*yci ; new_im = cr*yci + ci*ycr
                nc.vector.scalar_tensor_tensor(tr[0:fj, :], yci, nci[0:fj, j : j + 1],
                                               s1[0:fj, :], AL.mult, AL.add)
                nc.vector.scalar_tensor_tensor(yci, ycr, ci[0:fj, j : j + 1],
                                               s2[0:fj, :], AL.mult, AL.add)
                nc.vector.tensor_copy(ycr, tr[0:fj, :])

        # ---- write output: transpose back to (t,f), DMA out -------------
        for c in range(C):
            for p, dst in ((0, yr_d), (1, yi_d)):
                for ti, th in enumerate(TT_SIZES):
                    ot = outp.tile([128, F], FP, tag="ot")
                    for j in range(NJ):
                        fj = FSZ[j]
                        ps = psB.tile([128, 128], FP, tag="tp")
                        nc.tensor.transpose(
                            ps[0:th, 0:fj],
                            Y[c][p][0:fj, j * TL + ti * 128 : j * TL + ti * 128 + th],
                            ident[0:fj, 0:fj])
                        nc.scalar.copy(ot[0:th, 128 * j : 128 * j + fj],
                                       ps[0:th, 0:fj])
                    nc.sync.dma_start(dst[c, ti * 128 : ti * 128 + th, :],
                                      ot[0:th, :])


def _in_maps(inputs):
    data_real = np.asarray(inputs["data_real"], dtype=np.float32)
    data_imag = np.asarray(inputs["data_imag"], dtype=np.float32)
    W1 = np.asarray(inputs["W1"], dtype=np.float32)
    b1 = np.asarray(inputs["b1"], dtype=np.float32)
    W2 = np.asarray(inputs["W2"], dtype=np.float32)
    b2 = np.asarray(inputs["b2"], dtype=np.float32)
    in_maps = []
    for k in range(N_CORES):
        b, th = k // TSPLIT, k % TSPLIT
        sl = slice(th * TL, (th + 1) * TL)
        in_maps.append({
            "xr": np.ascontiguousarray(data_real[b, sl]),
            "xi": np.ascontiguousarray(data_imag[b, sl]),
            "W1": W1, "b1": b1, "W2": W2, "b2": b2,
        })
    return in_maps


def kernel(**inputs):
    if "nc" not in _CACHED:
        _CACHED["nc"] = _build()
    nc = _CACHED["nc"]
    in_maps = _in_maps(inputs)
    res = run_bass_kernel_spmd(nc, in_maps, list(range(N_CORES)))
    out = np.empty((C, B, T, F), dtype=np.complex64)
    for k in range(N_CORES):
        b, th = k // TSPLIT, k % TSPLIT
        r = res.results[k]["yr"]
        i = res.results[k]["yi"]
        out[:, b, th * TL : (th + 1) * TL, :] = r + 1j * i
    return out


if __name__ == "__main__":
    rng = np.random.default_rng(0)
    ins = {
        "data_real": rng.standard_normal((B, T, C, F), dtype=np.float32),
        "data_imag": rng.standard_normal((B, T, C, F), dtype=np.float32),
        "ilens": np.full((B,), T, dtype=np.int32),
        "W1": rng.standard_normal((F, U), dtype=np.float32) / np.sqrt(F),
        "b1": np.zeros((U,), dtype=np.float32),
        "W2": rng.standard_normal((U, F), dtype=np.float32) / np.sqrt(U),
        "b2": np.zeros((F,), dtype=np.float32),
    }
    out = kernel(**ins)
    print("kernel ran", out.shape, out.dtype, np.abs(out).mean())
